# revision 32
# baseline (speedup 1.0000x reference)
"""Trainium2 Bass kernel for nn_BatchGraphEncoder (gnn_message_passing).

Math note: the reference's segment softmax uses B unique segment ids
(groups of size 1), so alpha == exp(x-x)/1 == 1.0 bit-exactly for any
finite scores.  The output is therefore independent of the attention
inputs (w_i, w_j, w_k) and reduces to pure batch sums:

    out[:,   0:128] = sum_b h[b,:]      (broadcast over the N=512 rows)
    out[:, 128:256] = sum_b r[b,:]      (broadcast)
    out[:, 256:384] = sum_b t[b,:,:]    ([512, 128])

This is a memory-bound reduction over B=2048 dominated by reading t
(512 MB).  Strategy: shard B across the 8 cores (data parallel), reduce
over the local batch on-device, and sum the 8 tiny partials on the host.

Layout (8 KB descriptors): partition p = 32*(b%4) + q holds flat
columns [2048q, 2048q+2048) of batch rows congruent to b%4; each DMA
descriptor moves one 8 KB contiguous run (vs 2 KB when one row spans
all 128 partitions), cutting per-descriptor SDMA overhead.  The DVE
merges tiles into a [128, 4096] accumulator (two 2048-wide bh slots);
the final partial ships as [128, 2048] and the host folds the 4
partition groups (b%4) along with the cross-core sum.

Pipeline: one tensor_tensor merge per 2 MB sub-DMA, 5 x 4 MB pool
slots, the first tile split 4 x 1 MB so the DVE starts ~13us in, and a
1 MB-granularity taper carved from two pool slots as sub-range DMAs so
the stream end never starves on the pool window.

The h/r sums ride on the otherwise-idle TensorEngine: a stationary
matrix whose column j is all-ones places column-sums of the moving
operand into PSUM row j (rows 0/1 = sum_h/sum_r).

Load balancing: cores 4 and 6 of this machine lose ~6%/12% DMA
bandwidth, so they get smaller shards: rows [224, 240) are only loaded
when partition_id != 6, rows [240, 264) when partition_id not in
{4, 6} (the skipping cores' buffers hold stale finite data there; a
per-partition scalar mask gates those tiles' accumulator merges, and
h/r padding rows are zeros, which is exact for a sum).
"""

import numpy as np

B, N, D = 2048, 512, 128
NCORES = 8
FLAT = N * D                 # 65536 flattened (n, d) columns
BLK = 2048                   # flat columns per partition block (8 KB runs)
ACCW = 2 * BLK               # accumulator free width (2 bh slots)

B_FAST = 264
SIZES = [B_FAST] * NCORES
SIZES[4] = 240
SIZES[6] = 224
assert sum(SIZES) == B

_UA = [(r, 16, None) for r in range(0, 80, 16)]      # 5 warm-up 4 MB tiles
_C6 = [(224, 8, "c6"), (232, 8, "c6")]
_C46 = [(240, 8, "c46"), (248, 8, "c46"), (256, 8, "c46")]
_UB = [(r, 16, None) for r in range(80, 176, 16)]    # 6 big 4 MB tiles
_T8 = [(r, 8, None) for r in range(176, 200, 8)]     # 3 x 2 MB
TILE_PLAN = _UA + _C6 + _C46 + _UB + _T8
assert sum(nb for _, nb, _ in TILE_PLAN) == B_FAST - 24
# 1 MB-granularity taper carved from two further pool slots.
_TA = [(r, 4) for r in range(200, 216, 4)]           # 4 subs in slot A
_TB = [(r, 4) for r in range(216, 224, 4)]           # 2 subs in slot B

NBUFS = 4       # main-stream pool slots (4 MB each)
NCOND = 2       # conditional tiles get their own slots so their gpsimd
                # merges never gate main-pool slot recycling

_BUILT = None
# test.py can inject {"trace": True, ...} here; harness path leaves it empty.
RUN_KWARGS = {}
LAST_RESULTS = None


def _build():
    from concourse import bacc, tile, mybir

    f32 = mybir.dt.float32
    add = mybir.AluOpType.add
    nc = bacc.Bacc(
        "TRN2",
        target_bir_lowering=False,
        debug=False,
        enable_asserts=False,
        num_devices=NCORES,
    )
    t_in = nc.dram_tensor("t_shard", [B_FAST, FLAT], f32, kind="ExternalInput").ap()
    h_in = nc.dram_tensor("h_shard", [B_FAST, D], f32, kind="ExternalInput").ap()
    r_in = nc.dram_tensor("r_shard", [B_FAST, D], f32, kind="ExternalInput").ap()
    out_t = nc.dram_tensor("out_t_part", [128, BLK], f32, kind="ExternalOutput").ap()
    out_hr = nc.dram_tensor("out_hr_part", [2, D], f32, kind="ExternalOutput").ap()

    with tile.TileContext(nc) as tc:
        with (
            tc.tile_pool(name="wconst", bufs=1) as wpool,
            tc.tile_pool(name="loads", bufs=NBUFS) as loads,
            tc.tile_pool(name="conds", bufs=NCOND) as conds,
            tc.tile_pool(name="hr", bufs=6) as hrpool,
            tc.tile_pool(name="res", bufs=1) as res,
            tc.tile_pool(name="acc", bufs=1, space="PSUM") as ppool,
        ):
            W = wpool.tile([128, 256], f32)
            mask6 = wpool.tile([128, 1], f32)
            mask46 = wpool.tile([128, 1], f32)
            psum_hr = ppool.tile([128, D], f32)
            acc = res.tile([128, ACCW], f32)
            acc2 = res.tile([128, ACCW], f32)  # gpsimd-owned cond accumulator
            res_hr = res.tile([2, D], f32)
            skip_cond = {}
            gp_skip = {}

            def emit_setup_and_hr():
                nc.vector.memset(W[:], 0.0)
                nc.vector.memset(W[:, 128:129], 1.0)
                nc.vector.memset(acc2[:], 0.0)
                pid_gp = nc.gpsimd.partition_id()
                # skip predicates for the gpsimd cond-merge path: nonzero
                # on the core(s) whose cond DMAs are predicated off
                gp_skip["c6"] = pid_gp == 6
                gp_skip["c46"] = (pid_gp == 6) + (pid_gp == 4)
                pid_sync = nc.sync.partition_id()
                pid_act = nc.scalar.partition_id()
                skip_cond["c6"] = {
                    nc.sync: pid_sync != 6,
                    nc.scalar: pid_act != 6,
                }
                skip_cond["c46"] = {
                    nc.sync: (pid_sync != 6) * (pid_sync != 4),
                    nc.scalar: (pid_act != 6) * (pid_act != 4),
                }
                chunks = []
                for row, src in ((0, h_in), (1, r_in)):
                    for c0 in range(0, B_FAST, 128):
                        k = min(128, B_FAST - c0)
                        ht = hrpool.tile([128, D], f32)
                        nc.gpsimd.dma_start(ht[:k, :], src[c0 : c0 + k, :])
                        chunks.append((row, ht, k))
                for i, (row, ht, k) in enumerate(chunks):
                    nc.tensor.matmul(
                        psum_hr[:],
                        W[:k, 128 - row : 256 - row],
                        ht[:k, :],
                        start=(i == 0),
                        stop=(i == len(chunks) - 1),
                    )
                nc.vector.tensor_copy(res_hr[:], psum_hr[0:2, :])
                nc.sync.dma_start(out_hr[:], res_hr[:])

            ring_bytes = [0, 0]  # greedy byte-balance across the 2 HWDGE rings

            def pick_ring(k, nb):
                ring = (
                    (k % 2)
                    if ring_bytes[0] == ring_bytes[1]
                    else int(ring_bytes[1] < ring_bytes[0])
                )
                ring_bytes[ring] += nb
                return nc.sync if ring == 0 else nc.scalar

            def emit_dma(tl, off, b0, NB, cnd, k):
                # partition (b%4, q); 8 KB contiguous runs per descriptor
                fw = NB * 512
                src = t_in[b0 : b0 + NB, :].rearrange(
                    "(bh bl) (q c) -> (bl q) bh c", bl=4, c=BLK
                )
                dma = pick_ring(k, NB)
                dst = tl[:, off : off + fw].rearrange("p (b c) -> p b c", b=NB // 4)
                if cnd:
                    dma.dma_start(dst, src, cond=skip_cond[cnd][dma])
                else:
                    dma.dma_start(dst, src)

            for k, (b0, NB, cnd) in enumerate(TILE_PLAN):
                if k == 2:
                    emit_setup_and_hr()
                fw = NB * 512  # free width
                if cnd:
                    tl = conds.tile([128, 4096], f32, tag="cload")
                else:
                    tl = loads.tile([128, 8192], f32, tag="tload")
                if k == 0:
                    for i in range(4):
                        emit_dma(tl, i * BLK, b0 + 4 * i, 4, None, i)
                    nc.vector.tensor_copy(acc[:, :BLK], tl[:, :BLK])
                    nc.vector.tensor_copy(acc[:, BLK:ACCW], tl[:, BLK : 2 * BLK])
                    nc.vector.tensor_tensor(
                        acc[:, :BLK], acc[:, :BLK], tl[:, 2 * BLK : 3 * BLK], add
                    )
                    nc.vector.tensor_tensor(
                        acc[:, BLK:ACCW], acc[:, BLK:ACCW], tl[:, 3 * BLK :], add
                    )
                elif cnd:
                    # Conditional tiles merge on the otherwise-idle GpSimd
                    # engine into acc2, keeping ~23us off the DVE critical
                    # chain (DVE work ~148us vs ~155us stream window).  On
                    # the core(s) whose DMA is predicated off, a same-
                    # engine If-memset zeroes the stale slot first, so the
                    # merges are plain unconditional adds (gpsimd has no
                    # per-partition-scalar op).
                    half = fw // 2
                    emit_dma(tl, 0, b0, NB // 2, cnd, k)
                    emit_dma(tl, half, b0 + NB // 2, NB // 2, cnd, k + 1)
                    with tc.If(gp_skip[cnd]):
                        nc.gpsimd.memset(tl[:, :fw], 0.0)
                    for c0 in (0, half):
                        nc.gpsimd.tensor_tensor(
                            acc2[:, c0 : c0 + half],
                            acc2[:, c0 : c0 + half],
                            tl[:, c0 : c0 + half],
                            add,
                        )
                elif NB == 16:
                    emit_dma(tl, 0, b0, 8, None, k)
                    emit_dma(tl, ACCW, b0 + 8, 8, None, k + 1)
                    nc.vector.tensor_tensor(acc[:], acc[:], tl[:, :ACCW], add)
                    nc.vector.tensor_tensor(
                        acc[:], acc[:], tl[:, ACCW : 2 * ACCW], add
                    )
                else:
                    emit_dma(tl, 0, b0, NB, cnd, k)
                    nc.vector.tensor_tensor(acc[:, :fw], acc[:, :fw], tl[:, :fw], add)

            # --- taper: 1 MB sub-range DMAs into two further pool slots ---
            k = len(TILE_PLAN)
            tla = loads.tile([128, 8192], f32, tag="tload")
            for i, (b0, NB) in enumerate(_TA):
                emit_dma(tla, i * BLK, b0, NB, None, k + i)
            tlb = loads.tile([128, 8192], f32, tag="tload")
            for i, (b0, NB) in enumerate(_TB):
                emit_dma(tlb, i * BLK, b0, NB, None, k + 4 + i)

            # GpSimd folds its cond accumulator once its merges finish
            # (mid-stream), leaving the DVE one 2048-wide combine at the end.
            nc.gpsimd.tensor_tensor(
                acc2[:, :BLK], acc2[:, :BLK], acc2[:, BLK:ACCW], add
            )

            # bh slot 1 is final after the last NB>=8 tile; fold it while
            # the taper streams, then merge taper subs into slot 0.
            nc.vector.tensor_tensor(acc[:, :BLK], acc[:, :BLK], acc[:, BLK:ACCW], add)
            for i in range(len(_TA)):
                o = i * BLK
                nc.vector.tensor_tensor(
                    acc[:, :BLK], acc[:, :BLK], tla[:, o : o + BLK], add
                )
            for i in range(len(_TB)):
                o = i * BLK
                nc.vector.tensor_tensor(
                    acc[:, :BLK], acc[:, :BLK], tlb[:, o : o + BLK], add
                )

            # Fold in the cond accumulator and ship the [128, 2048] partial
            # in two halves, each half's output DMA overlapping the other
            # half's combine; the host folds the four b%4 partition groups.
            nc.vector.tensor_tensor(
                acc[:, : BLK // 2], acc[:, : BLK // 2], acc2[:, : BLK // 2], add
            )
            nc.sync.dma_start(out_t[:, : BLK // 2], acc[:, : BLK // 2])
            nc.vector.tensor_tensor(
                acc[:, BLK // 2 : BLK],
                acc[:, BLK // 2 : BLK],
                acc2[:, BLK // 2 : BLK],
                add,
            )
            nc.scalar.dma_start(out_t[:, BLK // 2 :], acc[:, BLK // 2 : BLK])

    nc.compile()
    return nc


def _get_built():
    global _BUILT
    if _BUILT is None:
        _BUILT = _build()
    return _BUILT


def kernel(h, r, t, w_i, w_j, w_k):
    global LAST_RESULTS
    from concourse import bass_utils

    nc = _get_built()
    t2 = np.ascontiguousarray(t, dtype=np.float32).reshape(B, FLAT)
    h = np.ascontiguousarray(h, dtype=np.float32)
    r = np.ascontiguousarray(r, dtype=np.float32)

    def pad(a, ncols):
        out = np.zeros((B_FAST, ncols), dtype=np.float32)
        out[: a.shape[0]] = a
        return out

    starts = np.concatenate([[0], np.cumsum(SIZES)])
    in_maps = []
    for c in range(NCORES):
        s, e = int(starts[c]), int(starts[c + 1])
        if e - s == B_FAST:
            in_maps.append({"t_shard": t2[s:e], "h_shard": h[s:e], "r_shard": r[s:e]})
        else:
            in_maps.append(
                {
                    "t_shard": pad(t2[s:e], FLAT),
                    "h_shard": pad(h[s:e], D),
                    "r_shard": pad(r[s:e], D),
                }
            )
    results = bass_utils.run_bass_kernel_spmd(
        nc, in_maps, core_ids=list(range(NCORES)), **RUN_KWARGS
    )
    LAST_RESULTS = results

    sum_t = np.zeros(FLAT, dtype=np.float64)
    sum_h = np.zeros(D, dtype=np.float64)
    sum_r = np.zeros(D, dtype=np.float64)
    for c in range(NCORES):
        part = results.results[c]["out_t_part"]  # [128, BLK]
        # partition p = 32*(b%4) + q -> flat columns [2048q, 2048q+2048)
        sum_t += part.reshape(4, 32, BLK).sum(axis=0).reshape(FLAT)
        sum_h += results.results[c]["out_hr_part"][0]
        sum_r += results.results[c]["out_hr_part"][1]

    out = np.empty((N, 3 * D), dtype=np.float32)
    out[:, 0:D] = sum_h.astype(np.float32)[None, :]
    out[:, D : 2 * D] = sum_r.astype(np.float32)[None, :]
    out[:, 2 * D :] = sum_t.astype(np.float32).reshape(N, D)
    return out


# revision 35
# speedup vs baseline: 1.0851x; 1.0851x over previous
"""Trainium2 Bass kernel for nn_BatchGraphEncoder (gnn_message_passing).

Math note: the reference's segment softmax uses B unique segment ids
(groups of size 1), so alpha == exp(x-x)/1 == 1.0 bit-exactly for any
finite scores.  The output is therefore independent of the attention
inputs (w_i, w_j, w_k) and reduces to pure batch sums:

    out[:,   0:128] = sum_b h[b,:]      (broadcast over the N=512 rows)
    out[:, 128:256] = sum_b r[b,:]      (broadcast)
    out[:, 256:384] = sum_b t[b,:,:]    ([512, 128])

This is a memory-bound reduction over B=2048 dominated by reading t
(512 MB).  Strategy: shard B across the 8 cores (data parallel), reduce
over the local batch on-device, and sum the 8 tiny partials on the host.

Layout (8 KB descriptors): partition p = 32*(b%4) + q holds flat
columns [2048q, 2048q+2048) of batch rows congruent to b%4; each DMA
descriptor moves one 8 KB contiguous run (vs 2 KB when one row spans
all 128 partitions), cutting per-descriptor SDMA overhead.  The DVE
merges tiles into a [128, 4096] accumulator (two 2048-wide bh slots);
the final partial ships as [128, 2048] and the host folds the 4
partition groups (b%4) along with the cross-core sum.

Pipeline: one tensor_tensor merge per 2 MB sub-DMA, 5 x 4 MB pool
slots, the first tile split 4 x 1 MB so the DVE starts ~13us in, and a
1 MB-granularity taper carved from two pool slots as sub-range DMAs so
the stream end never starves on the pool window.

The h/r sums ride on the otherwise-idle TensorEngine: a stationary
matrix whose column j is all-ones places column-sums of the moving
operand into PSUM row j (rows 0/1 = sum_h/sum_r).

Load balancing: cores 4 and 6 of this machine lose ~6%/12% DMA
bandwidth, so they get smaller shards: rows [224, 240) are only loaded
when partition_id != 6, rows [240, 264) when partition_id not in
{4, 6} (the skipping cores' buffers hold stale finite data there; a
per-partition scalar mask gates those tiles' accumulator merges, and
h/r padding rows are zeros, which is exact for a sum).
"""

import numpy as np

B, N, D = 2048, 512, 128
NCORES = 8
FLAT = N * D                 # 65536 flattened (n, d) columns
BLK = 2048                   # flat columns per partition block (8 KB runs)
ACCW = 2 * BLK               # accumulator free width (2 bh slots)

B_FAST = 264
SIZES = [B_FAST] * NCORES
SIZES[4] = 240
SIZES[6] = 224
assert sum(SIZES) == B

_UA = [(r, 16, None) for r in range(0, 80, 16)]      # 5 warm-up 4 MB tiles
_C6 = [(224, 8, "c6"), (232, 8, "c6")]
_C46 = [(240, 8, "c46"), (248, 8, "c46"), (256, 8, "c46")]
_UB = [(r, 16, None) for r in range(80, 176, 16)]    # 6 big 4 MB tiles
_T8 = [(r, 8, None) for r in range(176, 200, 8)]     # 3 x 2 MB
TILE_PLAN = _UA + _C6 + _C46 + _UB + _T8
assert sum(nb for _, nb, _ in TILE_PLAN) == B_FAST - 24
# 1 MB-granularity taper carved from two further pool slots.
_TA = [(r, 4) for r in range(200, 216, 4)]           # 4 subs in slot A
_TB = [(r, 4) for r in range(216, 224, 4)]           # 2 subs in slot B

NBUFS = 10      # 2 MB slots: finer WAR recycling decouples DMA issue
                # from DVE merge pace (same 20 MB total window)

_BUILT = None
# test.py can inject {"trace": True, ...} here; harness path leaves it empty.
RUN_KWARGS = {}
LAST_RESULTS = None


def _build():
    from concourse import bacc, tile, mybir

    f32 = mybir.dt.float32
    add = mybir.AluOpType.add
    nc = bacc.Bacc(
        "TRN2",
        target_bir_lowering=False,
        debug=False,
        enable_asserts=False,
        num_devices=NCORES,
    )
    t_in = nc.dram_tensor("t_shard", [B_FAST, FLAT], f32, kind="ExternalInput").ap()
    h_in = nc.dram_tensor("h_shard", [B_FAST, D], f32, kind="ExternalInput").ap()
    r_in = nc.dram_tensor("r_shard", [B_FAST, D], f32, kind="ExternalInput").ap()
    out_t = nc.dram_tensor("out_t_part", [128, BLK], f32, kind="ExternalOutput").ap()
    out_hr = nc.dram_tensor("out_hr_part", [2, D], f32, kind="ExternalOutput").ap()

    with tile.TileContext(nc) as tc:
        with (
            tc.tile_pool(name="wconst", bufs=1) as wpool,
            tc.tile_pool(name="loads", bufs=NBUFS) as loads,
            tc.tile_pool(name="hr", bufs=6) as hrpool,
            tc.tile_pool(name="res", bufs=1) as res,
            tc.tile_pool(name="acc", bufs=1, space="PSUM") as ppool,
        ):
            W = wpool.tile([128, 256], f32)
            mask6 = wpool.tile([128, 1], f32)
            mask46 = wpool.tile([128, 1], f32)
            psum_hr = ppool.tile([128, D], f32)
            acc = res.tile([128, ACCW], f32)
            res_hr = res.tile([2, D], f32)
            skip_cond = {}
            masks = {"c6": mask6, "c46": mask46}

            def emit_setup_and_hr():
                nc.vector.memset(W[:], 0.0)
                nc.vector.memset(W[:, 128:129], 1.0)
                nc.vector.memset(mask6[:], 1.0)
                nc.vector.memset(mask46[:], 1.0)
                pid_vec = nc.vector.partition_id()
                with tc.If(pid_vec == 6):
                    nc.vector.memset(mask6[:], 0.0)
                    nc.vector.memset(mask46[:], 0.0)
                with tc.If(pid_vec == 4):
                    nc.vector.memset(mask46[:], 0.0)
                pid_sync = nc.sync.partition_id()
                pid_act = nc.scalar.partition_id()
                skip_cond["c6"] = {
                    nc.sync: pid_sync != 6,
                    nc.scalar: pid_act != 6,
                }
                skip_cond["c46"] = {
                    nc.sync: (pid_sync != 6) * (pid_sync != 4),
                    nc.scalar: (pid_act != 6) * (pid_act != 4),
                }
                chunks = []
                for row, src in ((0, h_in), (1, r_in)):
                    for c0 in range(0, B_FAST, 128):
                        k = min(128, B_FAST - c0)
                        ht = hrpool.tile([128, D], f32)
                        nc.gpsimd.dma_start(ht[:k, :], src[c0 : c0 + k, :])
                        chunks.append((row, ht, k))
                for i, (row, ht, k) in enumerate(chunks):
                    nc.tensor.matmul(
                        psum_hr[:],
                        W[:k, 128 - row : 256 - row],
                        ht[:k, :],
                        start=(i == 0),
                        stop=(i == len(chunks) - 1),
                    )
                nc.vector.tensor_copy(res_hr[:], psum_hr[0:2, :])
                nc.sync.dma_start(out_hr[:], res_hr[:])

            ring_bytes = [0, 0]  # greedy byte-balance across the 2 HWDGE rings

            def pick_ring(k, nb):
                ring = (
                    (k % 2)
                    if ring_bytes[0] == ring_bytes[1]
                    else int(ring_bytes[1] < ring_bytes[0])
                )
                ring_bytes[ring] += nb
                return nc.sync if ring == 0 else nc.scalar

            def emit_dma(tl, off, b0, NB, cnd, k):
                # partition (b%4, q); 8 KB contiguous runs per descriptor
                fw = NB * 512
                src = t_in[b0 : b0 + NB, :].rearrange(
                    "(bh bl) (q c) -> (bl q) bh c", bl=4, c=BLK
                )
                dma = pick_ring(k, NB)
                dst = tl[:, off : off + fw].rearrange("p (b c) -> p b c", b=NB // 4)
                if cnd:
                    dma.dma_start(dst, src, cond=skip_cond[cnd][dma])
                else:
                    dma.dma_start(dst, src)

            LAST_T8 = len(TILE_PLAN) - 1
            for k, (b0, NB, cnd) in enumerate(TILE_PLAN):
                if k == 2:
                    emit_setup_and_hr()
                if k == 0:
                    # 4 x 1 MB subs across two slots: first merge at ~13us
                    for i in range(2):
                        tl = loads.tile([128, ACCW], f32, tag="tload")
                        emit_dma(tl, 0, b0 + 8 * i, 4, None, 2 * i)
                        emit_dma(tl, BLK, b0 + 8 * i + 4, 4, None, 2 * i + 1)
                        if i == 0:
                            nc.vector.tensor_copy(acc[:, :BLK], tl[:, :BLK])
                            nc.vector.tensor_copy(acc[:, BLK:ACCW], tl[:, BLK:])
                        else:
                            nc.vector.tensor_tensor(
                                acc[:, :BLK], acc[:, :BLK], tl[:, :BLK], add
                            )
                            nc.vector.tensor_tensor(
                                acc[:, BLK:ACCW], acc[:, BLK:ACCW], tl[:, BLK:], add
                            )
                elif cnd:
                    # two masked half-merges: acc = (sub * mask) + acc
                    tl = loads.tile([128, ACCW], f32, tag="tload")
                    emit_dma(tl, 0, b0, NB // 2, cnd, k)
                    emit_dma(tl, BLK, b0 + NB // 2, NB // 2, cnd, k + 1)
                    for c0 in (0, BLK):
                        nc.vector.scalar_tensor_tensor(
                            acc[:, c0 : c0 + BLK],
                            tl[:, c0 : c0 + BLK],
                            masks[cnd][:],
                            acc[:, c0 : c0 + BLK],
                            mybir.AluOpType.mult,
                            add,
                        )
                elif NB == 16:
                    for i in range(2):
                        tl = loads.tile([128, ACCW], f32, tag="tload")
                        emit_dma(tl, 0, b0 + 8 * i, 8, None, k + i)
                        nc.vector.tensor_tensor(acc[:], acc[:], tl[:], add)
                else:
                    tl = loads.tile([128, ACCW], f32, tag="tload")
                    emit_dma(tl, 0, b0, NB, None, k)
                    if k == LAST_T8:
                        # hoist the bh-slot-1 pre-fold off the tail chain:
                        # it only needs the previous 4096-wide merges, and
                        # this last tile merges slot-0-only.
                        nc.vector.tensor_tensor(
                            acc[:, :BLK], acc[:, :BLK], acc[:, BLK:ACCW], add
                        )
                        nc.vector.tensor_tensor(
                            acc[:, :BLK], acc[:, :BLK], tl[:, :BLK], add
                        )
                        nc.vector.tensor_tensor(
                            acc[:, :BLK], acc[:, :BLK], tl[:, BLK:ACCW], add
                        )
                    else:
                        nc.vector.tensor_tensor(acc[:], acc[:], tl[:], add)

            # --- taper: 1 MB sub-range DMAs into three further slots ---
            k = len(TILE_PLAN)
            tsubs = _TA + _TB
            tls = []
            for i in range(0, len(tsubs), 2):
                tl = loads.tile([128, ACCW], f32, tag="tload")
                emit_dma(tl, 0, tsubs[i][0], 4, None, k + i)
                emit_dma(tl, BLK, tsubs[i + 1][0], 4, None, k + i + 1)
                tls.append(tl)
            for tl in tls:
                nc.vector.tensor_tensor(acc[:, :BLK], acc[:, :BLK], tl[:, :BLK], add)
                nc.vector.tensor_tensor(
                    acc[:, :BLK], acc[:, :BLK], tl[:, BLK:ACCW], add
                )

            # Ship the [128, 2048] partial in two ring-overlapped halves;
            # the host folds the four b%4 partition groups.
            nc.sync.dma_start(out_t[:, :BLK // 2], acc[:, : BLK // 2])
            nc.scalar.dma_start(out_t[:, BLK // 2 :], acc[:, BLK // 2 : BLK])

    nc.compile()
    return nc


def _get_built():
    global _BUILT
    if _BUILT is None:
        _BUILT = _build()
    return _BUILT


def kernel(h, r, t, w_i, w_j, w_k):
    global LAST_RESULTS
    from concourse import bass_utils

    nc = _get_built()
    t2 = np.ascontiguousarray(t, dtype=np.float32).reshape(B, FLAT)
    h = np.ascontiguousarray(h, dtype=np.float32)
    r = np.ascontiguousarray(r, dtype=np.float32)

    def pad(a, ncols):
        out = np.zeros((B_FAST, ncols), dtype=np.float32)
        out[: a.shape[0]] = a
        return out

    starts = np.concatenate([[0], np.cumsum(SIZES)])
    in_maps = []
    for c in range(NCORES):
        s, e = int(starts[c]), int(starts[c + 1])
        if e - s == B_FAST:
            in_maps.append({"t_shard": t2[s:e], "h_shard": h[s:e], "r_shard": r[s:e]})
        else:
            in_maps.append(
                {
                    "t_shard": pad(t2[s:e], FLAT),
                    "h_shard": pad(h[s:e], D),
                    "r_shard": pad(r[s:e], D),
                }
            )
    results = bass_utils.run_bass_kernel_spmd(
        nc, in_maps, core_ids=list(range(NCORES)), **RUN_KWARGS
    )
    LAST_RESULTS = results

    sum_t = np.zeros(FLAT, dtype=np.float64)
    sum_h = np.zeros(D, dtype=np.float64)
    sum_r = np.zeros(D, dtype=np.float64)
    for c in range(NCORES):
        part = results.results[c]["out_t_part"]  # [128, BLK]
        # partition p = 32*(b%4) + q -> flat columns [2048q, 2048q+2048)
        sum_t += part.reshape(4, 32, BLK).sum(axis=0).reshape(FLAT)
        sum_h += results.results[c]["out_hr_part"][0]
        sum_r += results.results[c]["out_hr_part"][1]

    out = np.empty((N, 3 * D), dtype=np.float32)
    out[:, 0:D] = sum_h.astype(np.float32)[None, :]
    out[:, D : 2 * D] = sum_r.astype(np.float32)[None, :]
    out[:, 2 * D :] = sum_t.astype(np.float32).reshape(N, D)
    return out


# revision 38
# speedup vs baseline: 1.0936x; 1.0078x over previous
"""Trainium2 Bass kernel for nn_BatchGraphEncoder (gnn_message_passing).

Math note: the reference's segment softmax uses B unique segment ids
(groups of size 1), so alpha == exp(x-x)/1 == 1.0 bit-exactly for any
finite scores.  The output is therefore independent of the attention
inputs (w_i, w_j, w_k) and reduces to pure batch sums:

    out[:,   0:128] = sum_b h[b,:]      (broadcast over the N=512 rows)
    out[:, 128:256] = sum_b r[b,:]      (broadcast)
    out[:, 256:384] = sum_b t[b,:,:]    ([512, 128])

This is a memory-bound reduction over B=2048 dominated by reading t
(512 MB).  Strategy: shard B across the 8 cores (data parallel), reduce
over the local batch on-device, and sum the 8 tiny partials on the host.

Layout (8 KB descriptors): partition p = 32*(b%4) + q holds flat
columns [2048q, 2048q+2048) of batch rows congruent to b%4; each DMA
descriptor moves one 8 KB contiguous run (vs 2 KB when one row spans
all 128 partitions), cutting per-descriptor SDMA overhead.  The DVE
merges tiles into a [128, 4096] accumulator (two 2048-wide bh slots);
the final partial ships as [128, 2048] and the host folds the 4
partition groups (b%4) along with the cross-core sum.

Pipeline: one tensor_tensor merge per 2 MB sub-DMA, 5 x 4 MB pool
slots, the first tile split 4 x 1 MB so the DVE starts ~13us in, and a
1 MB-granularity taper carved from two pool slots as sub-range DMAs so
the stream end never starves on the pool window.

The h/r sums ride on the otherwise-idle TensorEngine: a stationary
matrix whose column j is all-ones places column-sums of the moving
operand into PSUM row j (rows 0/1 = sum_h/sum_r).

Load balancing: cores 4 and 6 of this machine lose ~6%/12% DMA
bandwidth, so they get smaller shards: rows [224, 240) are only loaded
when partition_id != 6, rows [240, 264) when partition_id not in
{4, 6} (the skipping cores' buffers hold stale finite data there; a
per-partition scalar mask gates those tiles' accumulator merges, and
h/r padding rows are zeros, which is exact for a sum).
"""

import numpy as np

B, N, D = 2048, 512, 128
NCORES = 8
FLAT = N * D                 # 65536 flattened (n, d) columns
BLK = 2048                   # flat columns per partition block (8 KB runs)
ACCW = 2 * BLK               # accumulator free width (2 bh slots)

B_FAST = 264
SIZES = [B_FAST] * NCORES
SIZES[4] = 240
SIZES[6] = 224
assert sum(SIZES) == B

_UA = [(r, 16, None) for r in range(0, 80, 16)]      # 5 warm-up 4 MB tiles
_C6 = [(224, 8, "c6"), (232, 8, "c6")]
_C46 = [(240, 8, "c46"), (248, 8, "c46"), (256, 8, "c46")]
_UB = [(r, 16, None) for r in range(80, 176, 16)]    # 6 big 4 MB tiles
_T8 = [(r, 8, None) for r in range(176, 200, 8)]     # 3 x 2 MB
# Conditional tiles interleave among the UB tiles (slot-calls >= 10 by
# then, so first-use SBUF is safe) so their ~4.5us masked-merge pairs
# fall into the DVE's per-pair wait gaps instead of forming a 22us
# contiguous block that backlogs the chain.
_CONDS = _C6 + _C46
TILE_PLAN = _UA + [
    t for pair in zip(_UB[:5], _CONDS) for t in pair
] + [_UB[5]] + _T8
assert sum(nb for _, nb, _ in TILE_PLAN) == B_FAST - 24
# 1 MB-granularity taper carved from two further pool slots.
_TA = [(r, 4) for r in range(200, 216, 4)]           # 4 subs in slot A
_TB = [(r, 4) for r in range(216, 224, 4)]           # 2 subs in slot B

NBUFS = 10      # 2 MB slots: finer WAR recycling decouples DMA issue
                # from DVE merge pace (same 20 MB total window)

_BUILT = None
# test.py can inject {"trace": True, ...} here; harness path leaves it empty.
RUN_KWARGS = {}
LAST_RESULTS = None


def _build():
    from concourse import bacc, tile, mybir

    f32 = mybir.dt.float32
    add = mybir.AluOpType.add
    nc = bacc.Bacc(
        "TRN2",
        target_bir_lowering=False,
        debug=False,
        enable_asserts=False,
        num_devices=NCORES,
    )
    t_in = nc.dram_tensor("t_shard", [B_FAST, FLAT], f32, kind="ExternalInput").ap()
    h_in = nc.dram_tensor("h_shard", [B_FAST, D], f32, kind="ExternalInput").ap()
    r_in = nc.dram_tensor("r_shard", [B_FAST, D], f32, kind="ExternalInput").ap()
    out_t = nc.dram_tensor("out_t_part", [128, BLK], f32, kind="ExternalOutput").ap()
    out_hr = nc.dram_tensor("out_hr_part", [2, D], f32, kind="ExternalOutput").ap()

    with tile.TileContext(nc) as tc:
        with (
            tc.tile_pool(name="wconst", bufs=1) as wpool,
            tc.tile_pool(name="loads", bufs=NBUFS) as loads,
            tc.tile_pool(name="hr", bufs=6) as hrpool,
            tc.tile_pool(name="res", bufs=1) as res,
            tc.tile_pool(name="acc", bufs=1, space="PSUM") as ppool,
        ):
            W = wpool.tile([128, 256], f32)
            mask6 = wpool.tile([128, 1], f32)
            mask46 = wpool.tile([128, 1], f32)
            psum_hr = ppool.tile([128, D], f32)
            acc = res.tile([128, ACCW], f32)
            res_hr = res.tile([2, D], f32)
            skip_cond = {}
            masks = {"c6": mask6, "c46": mask46}

            def emit_setup_and_hr():
                nc.vector.memset(W[:], 0.0)
                nc.vector.memset(W[:, 128:129], 1.0)
                nc.vector.memset(mask6[:], 1.0)
                nc.vector.memset(mask46[:], 1.0)
                pid_vec = nc.vector.partition_id()
                with tc.If(pid_vec == 6):
                    nc.vector.memset(mask6[:], 0.0)
                    nc.vector.memset(mask46[:], 0.0)
                with tc.If(pid_vec == 4):
                    nc.vector.memset(mask46[:], 0.0)
                pid_sync = nc.sync.partition_id()
                pid_act = nc.scalar.partition_id()
                skip_cond["c6"] = {
                    nc.sync: pid_sync != 6,
                    nc.scalar: pid_act != 6,
                }
                skip_cond["c46"] = {
                    nc.sync: (pid_sync != 6) * (pid_sync != 4),
                    nc.scalar: (pid_act != 6) * (pid_act != 4),
                }
                chunks = []
                for row, src in ((0, h_in), (1, r_in)):
                    for c0 in range(0, B_FAST, 128):
                        k = min(128, B_FAST - c0)
                        ht = hrpool.tile([128, D], f32)
                        nc.gpsimd.dma_start(ht[:k, :], src[c0 : c0 + k, :])
                        chunks.append((row, ht, k))
                for i, (row, ht, k) in enumerate(chunks):
                    nc.tensor.matmul(
                        psum_hr[:],
                        W[:k, 128 - row : 256 - row],
                        ht[:k, :],
                        start=(i == 0),
                        stop=(i == len(chunks) - 1),
                    )
                nc.vector.tensor_copy(res_hr[:], psum_hr[0:2, :])
                nc.sync.dma_start(out_hr[:], res_hr[:])

            ring_bytes = [0, 0]  # greedy byte-balance across the 2 HWDGE rings

            def pick_ring(k, nb):
                ring = (
                    (k % 2)
                    if ring_bytes[0] == ring_bytes[1]
                    else int(ring_bytes[1] < ring_bytes[0])
                )
                ring_bytes[ring] += nb
                return nc.sync if ring == 0 else nc.scalar

            def emit_dma(tl, off, b0, NB, cnd, k):
                # partition (b%4, q); 8 KB contiguous runs per descriptor
                fw = NB * 512
                src = t_in[b0 : b0 + NB, :].rearrange(
                    "(bh bl) (q c) -> (bl q) bh c", bl=4, c=BLK
                )
                dma = pick_ring(k, NB)
                dst = tl[:, off : off + fw].rearrange("p (b c) -> p b c", b=NB // 4)
                if cnd:
                    dma.dma_start(dst, src, cond=skip_cond[cnd][dma])
                else:
                    dma.dma_start(dst, src)

            LAST_T8 = len(TILE_PLAN) - 1
            for k, (b0, NB, cnd) in enumerate(TILE_PLAN):
                if k == 2:
                    emit_setup_and_hr()
                if k == 0:
                    # 4 x 1 MB subs across two slots: first merge at ~13us
                    for i in range(2):
                        tl = loads.tile([128, ACCW], f32, tag="tload")
                        emit_dma(tl, 0, b0 + 8 * i, 4, None, 2 * i)
                        emit_dma(tl, BLK, b0 + 8 * i + 4, 4, None, 2 * i + 1)
                        if i == 0:
                            nc.vector.tensor_copy(acc[:, :BLK], tl[:, :BLK])
                            nc.vector.tensor_copy(acc[:, BLK:ACCW], tl[:, BLK:])
                        else:
                            nc.vector.tensor_tensor(
                                acc[:, :BLK], acc[:, :BLK], tl[:, :BLK], add
                            )
                            nc.vector.tensor_tensor(
                                acc[:, BLK:ACCW], acc[:, BLK:ACCW], tl[:, BLK:], add
                            )
                elif cnd:
                    # two masked half-merges: acc = (sub * mask) + acc
                    tl = loads.tile([128, ACCW], f32, tag="tload")
                    emit_dma(tl, 0, b0, NB // 2, cnd, k)
                    emit_dma(tl, BLK, b0 + NB // 2, NB // 2, cnd, k + 1)
                    for c0 in (0, BLK):
                        nc.vector.scalar_tensor_tensor(
                            acc[:, c0 : c0 + BLK],
                            tl[:, c0 : c0 + BLK],
                            masks[cnd][:],
                            acc[:, c0 : c0 + BLK],
                            mybir.AluOpType.mult,
                            add,
                        )
                elif NB == 16:
                    # two 2 MB slot-tiles, each as 2 x 1 MB sub-DMAs: a
                    # 1 MB sub's completion trails its first byte by only
                    # ~4.7us (vs ~9.4 for 2 MB under 2-ring interleave)
                    for i in range(2):
                        tl = loads.tile([128, ACCW], f32, tag="tload")
                        emit_dma(tl, 0, b0 + 8 * i, 4, None, k + i)
                        emit_dma(tl, BLK, b0 + 8 * i + 4, 4, None, k + i + 1)
                        nc.vector.tensor_tensor(
                            acc[:, :BLK], acc[:, :BLK], tl[:, :BLK], add
                        )
                        nc.vector.tensor_tensor(
                            acc[:, BLK:ACCW], acc[:, BLK:ACCW], tl[:, BLK:], add
                        )
                else:
                    tl = loads.tile([128, ACCW], f32, tag="tload")
                    emit_dma(tl, 0, b0, NB // 2, None, k)
                    emit_dma(tl, BLK, b0 + NB // 2, NB // 2, None, k + 1)
                    if k == LAST_T8:
                        # hoist the bh-slot-1 pre-fold off the tail chain:
                        # it only needs the previous 4096-wide merges, and
                        # this last tile merges slot-0-only.
                        nc.vector.tensor_tensor(
                            acc[:, :BLK], acc[:, :BLK], acc[:, BLK:ACCW], add
                        )
                        nc.vector.tensor_tensor(
                            acc[:, :BLK], acc[:, :BLK], tl[:, :BLK], add
                        )
                        nc.vector.tensor_tensor(
                            acc[:, :BLK], acc[:, :BLK], tl[:, BLK:ACCW], add
                        )
                    else:
                        nc.vector.tensor_tensor(
                            acc[:, :BLK], acc[:, :BLK], tl[:, :BLK], add
                        )
                        nc.vector.tensor_tensor(
                            acc[:, BLK:ACCW], acc[:, BLK:ACCW], tl[:, BLK:], add
                        )

            # --- taper: 1 MB sub-range DMAs into three further slots ---
            k = len(TILE_PLAN)
            tsubs = _TA + _TB
            tls = []
            for i in range(0, len(tsubs), 2):
                tl = loads.tile([128, ACCW], f32, tag="tload")
                emit_dma(tl, 0, tsubs[i][0], 4, None, k + i)
                emit_dma(tl, BLK, tsubs[i + 1][0], 4, None, k + i + 1)
                tls.append(tl)
            for tl in tls:
                nc.vector.tensor_tensor(acc[:, :BLK], acc[:, :BLK], tl[:, :BLK], add)
                nc.vector.tensor_tensor(
                    acc[:, :BLK], acc[:, :BLK], tl[:, BLK:ACCW], add
                )

            # Ship the [128, 2048] partial in two ring-overlapped halves;
            # the host folds the four b%4 partition groups.
            nc.sync.dma_start(out_t[:, :BLK // 2], acc[:, : BLK // 2])
            nc.scalar.dma_start(out_t[:, BLK // 2 :], acc[:, BLK // 2 : BLK])

    nc.compile()
    return nc


def _get_built():
    global _BUILT
    if _BUILT is None:
        _BUILT = _build()
    return _BUILT


def kernel(h, r, t, w_i, w_j, w_k):
    global LAST_RESULTS
    from concourse import bass_utils

    nc = _get_built()
    t2 = np.ascontiguousarray(t, dtype=np.float32).reshape(B, FLAT)
    h = np.ascontiguousarray(h, dtype=np.float32)
    r = np.ascontiguousarray(r, dtype=np.float32)

    def pad(a, ncols):
        out = np.zeros((B_FAST, ncols), dtype=np.float32)
        out[: a.shape[0]] = a
        return out

    starts = np.concatenate([[0], np.cumsum(SIZES)])
    in_maps = []
    for c in range(NCORES):
        s, e = int(starts[c]), int(starts[c + 1])
        if e - s == B_FAST:
            in_maps.append({"t_shard": t2[s:e], "h_shard": h[s:e], "r_shard": r[s:e]})
        else:
            in_maps.append(
                {
                    "t_shard": pad(t2[s:e], FLAT),
                    "h_shard": pad(h[s:e], D),
                    "r_shard": pad(r[s:e], D),
                }
            )
    results = bass_utils.run_bass_kernel_spmd(
        nc, in_maps, core_ids=list(range(NCORES)), **RUN_KWARGS
    )
    LAST_RESULTS = results

    sum_t = np.zeros(FLAT, dtype=np.float64)
    sum_h = np.zeros(D, dtype=np.float64)
    sum_r = np.zeros(D, dtype=np.float64)
    for c in range(NCORES):
        part = results.results[c]["out_t_part"]  # [128, BLK]
        # partition p = 32*(b%4) + q -> flat columns [2048q, 2048q+2048)
        sum_t += part.reshape(4, 32, BLK).sum(axis=0).reshape(FLAT)
        sum_h += results.results[c]["out_hr_part"][0]
        sum_r += results.results[c]["out_hr_part"][1]

    out = np.empty((N, 3 * D), dtype=np.float32)
    out[:, 0:D] = sum_h.astype(np.float32)[None, :]
    out[:, D : 2 * D] = sum_r.astype(np.float32)[None, :]
    out[:, 2 * D :] = sum_t.astype(np.float32).reshape(N, D)
    return out
